# revision 3
# baseline (speedup 1.0000x reference)
"""Trainium2 Bass kernel for nn_Attention_20315195310831 (v2).

Fused attention block: q/k/v projections, per-head RMS-norm on q/k, masked
softmax with per-head gating, value residual, output projection.

Sharding over 8 NeuronCores: core = 4*b + grp handles batch b and heads
[4*grp, 4*grp+4). Each core computes its partial (attn_out + vx) @ Wo_slice;
the host sums the 4 partials per batch.

v2 changes vs the bf16 baseline:
- Projections run as fp8 DoubleRow matmuls (contraction 256/pass, 0.5
  cyc/col) with 3-term error compensation: x8@W8 + dx5@W8 + x8@dW5, where
  x8/W8 are e4m3 and the deltas are e5m2 (covers the small-residual range).
- Scores run as fp8 DoubleRow with the slot pair carrying (k8, k_lo)
  compensation levels at full D=64 contraction; q is single-quantized e4m3
  and its slot pair is a stride-0 broadcast_to view.
- Engine rebalance: Square/normalize drains on DVE (GPSIMD cannot touch
  PSUM), mask multiplies + fp8 k packing on Pool, AT transposes via
  SBUF->SBUF DMA transpose, softmax-denominator reciprocals batched.
- Software pipelining: projections for token-quarter nk are emitted
  interleaved with attention for q-tile j=nk-1, so the exp stream (the
  Activation-engine bottleneck) starts early and never starves.

PSUM (8 banks): S pool 2x[128,1024] (scores + AV accumulators, the AV tile
uses both its banks' independent zero-regions for the two head chains),
P pool 2x[128,512] (projection chains + output-projection accumulation),
A pool 2x (rms row-reduce / rms broadcast / v-transposes).
"""

import sys

sys.path.insert(0, "/opt/trn_rl_repo")

import ml_dtypes
import numpy as np

B, T, C = 2, 2048, 1024
H, D = 16, 64
EPS = 1e-5
SCALE = 1.0 / 8.0  # 1/sqrt(D)
NCORES = 8
HPC = 4  # heads per core
NG = 2  # head-pair groups per core
C2 = 4  # 256-row contraction chunks
QT = 4  # q tiles of 512
QW = 512
TBLK = T // 128
BF16 = ml_dtypes.bfloat16
FP8 = ml_dtypes.float8_e4m3
FP8E5 = ml_dtypes.float8_e5m2

_CACHE = {}
USE_DMA_TRANSPOSE = True


def _analyze_mask(mask01):
    """mask01: bool [T, T], mask01[q, k] True = attend.  (unchanged from v1)"""
    pat_index = {}
    patterns = []

    def pat_id(block_qk):
        add = np.where(block_qk.T, 1.0, 0.0).astype(np.float32)
        key = add.tobytes()
        if key not in pat_index:
            pat_index[key] = len(patterns)
            patterns.append(add)
        return pat_index[key]

    plan = []
    for j in range(QT):
        entries = []
        for kb in range(TBLK):
            qbs = []
            for qb in range(4):
                blk = mask01[
                    (4 * j + qb) * 128 : (4 * j + qb + 1) * 128,
                    kb * 128 : (kb + 1) * 128,
                ]
                qbs.append(blk)
            anyb = [b.any() for b in qbs]
            if not any(anyb):
                continue
            lo = anyb.index(True)
            hi = 4 - anyb[::-1].index(True)
            entries.append([kb, lo, hi, qbs])
        if entries:
            ulo = min(e[1] for e in entries)
            uhi = max(e[2] for e in entries)
            entries[0][1] = ulo
            entries[0][2] = uhi
        final = []
        for kb, lo, hi, qbs in entries:
            subs = []
            for qb in range(lo, hi):
                if not qbs[qb].all():
                    subs.append((qb, pat_id(qbs[qb])))
            final.append((kb, lo * 128, hi * 128, subs))
        plan.append(final)

    if not patterns:
        patterns.append(np.zeros((128, 128), np.float32))
    return plan, np.stack(patterns)


def _build_program(plan, npat, neg_bias):
    import concourse.mybir as mybir
    import concourse.tile as tile
    from concourse import bacc

    f32 = mybir.dt.float32
    bf16 = mybir.dt.bfloat16
    fp8 = mybir.dt.float8e4
    fp8e5 = mybir.dt.float8e5
    AF = mybir.ActivationFunctionType
    OP = mybir.AluOpType
    DR = mybir.MatmulPerfMode.DoubleRow

    nc = bacc.Bacc(
        "TRN2",
        target_bir_lowering=False,
        debug=False,
        enable_asserts=False,
        num_devices=NCORES,
    )

    # host layouts:
    #  x8/xd: [128, c2(4), slot(2), T]  (slot = K-row pair for DoubleRow)
    #  w8/wd: [128, c2(4), slot(2), 256hd]
    x8_d = nc.dram_tensor("x8", [128, C2 * 2 * T], fp8, kind="ExternalInput").ap()
    xd_d = nc.dram_tensor("xd", [128, C2 * 2 * T], fp8e5, kind="ExternalInput").ap()
    w8_d = {}
    wd_d = {}
    for nm in ("wq", "wk", "wv"):
        w8_d[nm] = nc.dram_tensor(f"{nm}8", [128, C2 * 2 * 256], fp8,
                                  kind="ExternalInput").ap()
        wd_d[nm] = nc.dram_tensor(f"{nm}d", [128, C2 * 2 * 256], fp8e5,
                                  kind="ExternalInput").ap()
    wo_d = nc.dram_tensor("wo", [128, 2048], bf16, kind="ExternalInput").ap()
    wqc_d = nc.dram_tensor("wq_col", [128, 1], f32, kind="ExternalInput").ap()
    wkc_d = nc.dram_tensor("wk_col", [128, 1], f32, kind="ExternalInput").ap()
    sel2_d = nc.dram_tensor("sel2", [128, 2], bf16, kind="ExternalInput").ap()
    selT4_d = nc.dram_tensor("selT4", [34, 512], bf16, kind="ExternalInput").ap()
    ident_d = nc.dram_tensor("ident128", [128, 128], bf16, kind="ExternalInput").ap()
    pats_d = nc.dram_tensor("pats", [128, 128 * npat], bf16, kind="ExternalInput").ap()
    out_d = nc.dram_tensor("out", [T, C], bf16, kind="ExternalOutput").ap()

    x8v = x8_d.rearrange("p (c s t) -> p c s t", c=C2, s=2)
    xdv = xd_d.rearrange("p (c s t) -> p c s t", c=C2, s=2)
    w8v = {nm: w8_d[nm].rearrange("p (c s h) -> p c s h", c=C2, s=2)
           for nm in w8_d}
    wdv = {nm: wd_d[nm].rearrange("p (c s h) -> p c s h", c=C2, s=2)
           for nm in wd_d}

    with tile.TileContext(nc) as tc, \
         nc.allow_low_precision(reason="fp8/bf16 staging validated against fp32 reference"):
        with tc.tile_pool(name="pers", bufs=1) as pers, \
             tc.tile_pool(name="rot", bufs=4) as rot, \
             tc.tile_pool(name="ptp", bufs=34) as ptp, \
             tc.tile_pool(name="obp", bufs=4) as obp, \
             tc.tile_pool(name="psS", bufs=2, space="PSUM") as psS, \
             tc.tile_pool(name="psP", bufs=2, space="PSUM") as psP, \
             tc.tile_pool(name="psA", bufs=2, space="PSUM") as psA:

            # ---- persistent tiles
            x8t = [pers.tile([128, 2, T], fp8, tag=f"x8_{c}", name=f"x8_{c}")
                   for c in range(C2)]
            xdt = [pers.tile([128, 2, T], fp8e5, tag=f"xd_{c}", name=f"xd_{c}")
                   for c in range(C2)]
            w8 = {nm: pers.tile([128, C2, 2, 256], fp8, tag=f"{nm}8sb",
                                name=f"{nm}8sb") for nm in ("wq", "wk", "wv")}
            wd = {nm: pers.tile([128, C2, 2, 256], fp8e5, tag=f"{nm}dsb",
                                name=f"{nm}dsb") for nm in ("wq", "wk", "wv")}
            wo_sb = pers.tile([128, 2048], bf16, tag="wo_sb", name="wo_sb")
            q8T = [pers.tile([128, T], fp8, tag=f"q8T{g}", name=f"q8T{g}")
                   for g in range(NG)]
            kDR = [pers.tile([128, 2, T], fp8, tag=f"kDR{g}", name=f"kDR{g}")
                   for g in range(NG)]
            vT = [pers.tile([128, T], bf16, tag=f"vT{g}", name=f"vT{g}")
                  for g in range(NG)]
            vaug = [pers.tile([128, 65 * HPC], bf16, tag=f"vaug{kb}",
                              name=f"vaug{kb}") for kb in range(TBLK)]
            AT = [pers.tile([128, T], bf16, tag=f"AT{g}", name=f"AT{g}")
                  for g in range(NG)]
            AT_q = [pers.tile([128, 256], bf16, tag=f"ATq{qbl}", name=f"ATq{qbl}")
                    for qbl in range(4)]
            wq_col = pers.tile([128, 1], f32, tag="wq_col_sb", name="wq_col_sb")
            wk_col = pers.tile([128, 1], f32, tag="wk_col_sb", name="wk_col_sb")
            sel2 = pers.tile([128, 2], bf16, tag="sel2_sb", name="sel2_sb")
            selT4 = pers.tile([34, 512], bf16, tag="selT4_sb", name="selT4_sb")
            ident = pers.tile([128, 128], bf16, tag="ident_sb", name="ident_sb")
            pats = pers.tile([128, 128 * npat], bf16, tag="pats_sb", name="pats_sb")
            eps_col = pers.tile([128, 1], f32, tag="eps_col", name="eps_col")
            nb_col = pers.tile([128, 1], f32, tag="nb_col", name="nb_col")
            one_col = pers.tile([128, 1], bf16, tag="one_col", name="one_col")
            nc.vector.memset(eps_col, EPS)
            nc.vector.memset(nb_col, neg_bias)
            nc.vector.memset(one_col, 1.0)

            for kb in range(TBLK):
                for h in range(HPC):
                    nc.gpsimd.tensor_copy(vaug[kb][:, 65 * h + 64 : 65 * h + 65],
                                          one_col)

            # ---- DMA staging plan -----------------------------------------
            def dma_batch(nk):
                if nk == 0:
                    nc.sync.dma_start(sel2, sel2_d)
                    nc.sync.dma_start(selT4, selT4_d)
                    nc.sync.dma_start(wq_col, wqc_d)
                    nc.sync.dma_start(wk_col, wkc_d)
                    nc.sync.dma_start(w8["wq"].rearrange("p a b c -> p (a b c)"), w8_d["wq"])
                    nc.sync.dma_start(w8["wk"].rearrange("p a b c -> p (a b c)"), w8_d["wk"])
                    for c in range(C2):
                        nc.sync.dma_start(x8t[c][:, :, 0:512], x8v[:, c, :, 0:512])
                    nc.sync.dma_start(wd["wq"].rearrange("p a b c -> p (a b c)"), wd_d["wq"])
                    nc.sync.dma_start(wd["wk"].rearrange("p a b c -> p (a b c)"), wd_d["wk"])
                    for c in range(C2):
                        nc.sync.dma_start(xdt[c][:, :, 0:512], xdv[:, c, :, 0:512])
                    nc.sync.dma_start(w8["wv"].rearrange("p a b c -> p (a b c)"), w8_d["wv"])
                    nc.sync.dma_start(wd["wv"].rearrange("p a b c -> p (a b c)"), wd_d["wv"])
                    nc.sync.dma_start(ident, ident_d)
                    nc.sync.dma_start(pats, pats_d)
                else:
                    cs = slice(512 * nk, 512 * (nk + 1))
                    for c in range(C2):
                        nc.sync.dma_start(x8t[c][:, :, cs], x8v[:, c, :, cs])
                    for c in range(C2):
                        nc.sync.dma_start(xdt[c][:, :, cs], xdv[:, c, :, cs])
                    if nk == 1:
                        nc.sync.dma_start(wo_sb, wo_d)

            # ---- projection chain for one (nm, g, nk) ---------------------
            def proj_chain(nm, g, nk):
                cs = slice(512 * nk, 512 * (nk + 1))
                hs = slice(128 * g, 128 * (g + 1))
                pp = psP.tile([128, 512], f32, tag="P", name=f"pp_{nm}{g}{nk}")
                n = 0
                for c in range(C2):
                    nc.tensor.matmul(pp, w8[nm][:, c, :, hs], x8t[c][:, :, cs],
                                     start=(n == 0), stop=False, perf_mode=DR)
                    n += 1
                for c in range(C2):
                    nc.tensor.matmul(pp, w8[nm][:, c, :, hs], xdt[c][:, :, cs],
                                     start=False, stop=False, perf_mode=DR)
                    n += 1
                for c in range(C2):
                    nc.tensor.matmul(pp, wd[nm][:, c, :, hs], x8t[c][:, :, cs],
                                     start=False, stop=(c == C2 - 1), perf_mode=DR)
                    n += 1
                return pp

            def rms_stage1(nm, g, nk, pp, st_all):
                """square (Pool) + row-reduce into the shared st tile.

                pp is released after the bf16 copy; the copy feeds both the
                square and the later normalize stt (keeps stt off PSUM)."""
                idx = 2 * g + (1 if nm == "wk" else 0)
                ppc = rot.tile([128, 512], bf16, tag=f"ppc{idx}",
                               name="ppc", bufs=2)
                nc.vector.tensor_copy(ppc, pp)
                sq = rot.tile([128, 512], bf16, tag="sq", name="sq", bufs=4)
                nc.gpsimd.tensor_tensor(sq, ppc, ppc, OP.mult)
                nc.tensor.matmul(st_all[32 * idx : 32 * idx + 2], sel2, sq,
                                 start=True, stop=True,
                                 tile_position=(0, 32 * idx))
                return ppc

            def rms_stage2(nk, g, st_all, ppcs):
                """one ln+exp over this group's (q,k) rms rows, then per-chain
                broadcast + fp8 normalize (Pool, all-SBUF).

                1/rms = exp(-0.5*ln(ms/D + eps)): ln and exp share the exp
                activation table, so the attention exp stream never reloads
                ACT tables (sqrt would force a reload per call)."""
                cs = slice(512 * nk, 512 * (nk + 1))
                base = 64 * g
                lnr = rot.tile([34, 512], bf16, tag="lnr", name="lnr", bufs=2)
                nc.scalar.activation(lnr, st_all[base : base + 34], AF.Ln,
                                     bias=eps_col[0:34], scale=1.0 / D)
                rinv = rot.tile([34, 512], bf16, tag="rinv", name="rinv",
                                bufs=2)
                nc.scalar.activation(rinv, lnr, AF.Exp, scale=-0.5)
                for nm in ("wq", "wk"):
                    idx = 2 * g + (1 if nm == "wk" else 0)
                    mm = psA.tile([128, 512], f32, tag="A", name="mm_ps")
                    nc.tensor.matmul(
                        mm, selT4[0:34, 128 * idx : 128 * (idx + 1)],
                        rinv, start=True, stop=True)
                    mrbc = rot.tile([128, 512], bf16, tag="mrbc",
                                    name="mrbc", bufs=2)
                    nc.vector.tensor_copy(mrbc, mm)
                    ppc = ppcs[idx]
                    if nm == "wq":
                        nc.gpsimd.tensor_tensor(
                            q8T[g][:, cs], ppc, mrbc, OP.mult)
                    else:
                        ktmp = rot.tile([128, 512], bf16, tag="ktmp",
                                        name="ktmp", bufs=2)
                        nc.gpsimd.tensor_tensor(
                            ktmp, ppc, mrbc, OP.mult)
                        nc.gpsimd.tensor_copy(kDR[g][:, 0, cs], ktmp)
                        nc.gpsimd.tensor_tensor(kDR[g][:, 1, cs], ktmp,
                                                kDR[g][:, 0, cs],
                                                OP.subtract)

            def v_epilogue(g, nk, pp):
                cs = slice(512 * nk, 512 * (nk + 1))
                nc.vector.tensor_copy(vT[g][:, cs], pp)
                for kb in range(4 * nk, 4 * nk + 4):
                    pv = psA.tile([128, 128], bf16, tag="A", name="vtr_ps")
                    nc.tensor.transpose(pv, vT[g][:, 128 * kb : 128 * (kb + 1)],
                                        ident)
                    dst = vaug[kb].rearrange("p (h c) -> p h c", h=HPC)[
                        :, 2 * g : 2 * g + 2, 0:64]
                    nc.vector.tensor_copy(
                        dst, pv.rearrange("p (h c) -> p h c", h=2))

            # ---- attention machinery --------------------------------------
            pending_oproj = []

            def emit_oproj(count=99):
                while pending_oproj and count > 0:
                    count -= 1
                    tb = pending_oproj.pop(0)
                    for nn in range(2):
                        po = psP.tile([128, 512], f32, tag="P", name="po_ps")
                        for cg in range(NG):
                            nc.tensor.matmul(
                                po,
                                AT[cg][:, 128 * tb : 128 * (tb + 1)],
                                wo_sb[:, 1024 * cg + 512 * nn :
                                      1024 * cg + 512 * (nn + 1)],
                                start=(cg == 0),
                                stop=(cg == NG - 1),
                            )
                        ob = obp.tile([128, 512], bf16, tag="ob", name="ob")
                        nc.vector.tensor_copy(ob, po)
                        nc.sync.dma_start(
                            out_d[128 * tb : 128 * (tb + 1),
                                  512 * nn : 512 * (nn + 1)],
                            ob,
                        )

            def attn_tile(j):
                """emit attention for q-tile j as a list of chunks; caller
                interleaves the chunks with projection chains."""
                entries = plan[j]
                if not entries:
                    return []
                nent = len(entries)
                last_for_qb = {}
                for ei, (kb, ql, qh, subs) in enumerate(entries):
                    for qbl in range(ql // 128, qh // 128):
                        last_for_qb[qbl] = ei
                pts = {}
                av_done = set()

                def emit_av(g, qbl):
                    av_done.add((g, qbl))
                    o_q = [psP.tile([128, 512], f32, tag="P", name="o_q")
                           for _ in range(2)]
                    started = False
                    for ei, (kb, ql, qh, subs) in enumerate(entries):
                        if not (ql <= 128 * qbl < qh):
                            continue
                        for hl in range(2):
                            h = 2 * g + hl
                            nc.tensor.matmul(
                                o_q[hl][:, 0:65],
                                pts[(g, ei)][
                                    :, 512 * hl + 128 * qbl :
                                    512 * hl + 128 * (qbl + 1)],
                                vaug[kb][:, 65 * h : 65 * (h + 1)],
                                start=not started,
                                stop=(ei == last_for_qb[qbl]),
                            )
                        started = True
                    minv2 = rot.tile([128, 2], f32, tag="mv", name="minv2",
                                     bufs=4)
                    for hl in range(2):
                        nc.vector.reciprocal(minv2[:, hl : hl + 1],
                                             o_q[hl][:, 64:65])
                    for hl in range(2):
                        nc.vector.scalar_tensor_tensor(
                            AT_q[qbl][:, 128 * g + 64 * hl :
                                      128 * g + 64 * (hl + 1)],
                            o_q[hl][:, 0:64],
                            minv2[:, hl : hl + 1],
                            vaug[4 * j + qbl][
                                :, 65 * (2 * g + hl) : 65 * (2 * g + hl) + 64],
                            OP.mult,
                            OP.add,
                        )
                    if j == QT - 1:
                        emit_oproj(1)
                    if g == NG - 1:
                        tb = 4 * j + qbl
                        if USE_DMA_TRANSPOSE and j < QT - 1:
                            for cg in range(NG):
                                nc.sync.dma_start_transpose(
                                    AT[cg][:, 128 * tb : 128 * (tb + 1)],
                                    AT_q[qbl][:, 128 * cg : 128 * (cg + 1)],
                                )
                        else:
                            for cg in range(NG):
                                tps = psA.tile([128, 128], bf16, tag="A",
                                               name="tps")
                                nc.tensor.transpose(
                                    tps, AT_q[qbl][:, 128 * cg : 128 * (cg + 1)],
                                    ident)
                                nc.vector.tensor_copy(
                                    AT[cg][:, 128 * tb : 128 * (tb + 1)], tps)
                        if j == QT - 1:
                            pending_oproj.append(tb)
                            emit_oproj(1)

                chunks = []
                av_cadence = max(1, nent // 4)

                def score_chunk(g, eis):
                    def run():
                        for ei in eis:
                            kb, ql, qh, subs = entries[ei]
                            w = qh - ql
                            s_ps = psS.tile([128, 1024], f32, tag="S",
                                            name="s_ps")
                            for hl in range(2):
                                nc.tensor.matmul(
                                    s_ps[:, 512 * hl + ql : 512 * hl + qh],
                                    kDR[g][64 * hl : 64 * (hl + 1), :,
                                           128 * kb : 128 * (kb + 1)],
                                    q8T[g][64 * hl : 64 * (hl + 1),
                                           512 * j + ql : 512 * j + qh]
                                    .unsqueeze(1).broadcast_to([64, 2, w]),
                                    start=True,
                                    stop=True,
                                    perf_mode=DR,
                                    tile_position=(64 * hl, 0),
                                )
                            pt = ptp.tile([128, 1024], bf16, tag="PT", name="pt")
                            s3 = s_ps.rearrange("p (h w) -> p h w", h=2)
                            p3 = pt.rearrange("p (h w) -> p h w", h=2)
                            nc.scalar.activation(
                                p3[:, :, ql:qh], s3[:, :, ql:qh], AF.Exp,
                                bias=nb_col, scale=SCALE)
                            for hl in range(2):
                                for qbl, pidx in subs:
                                    bs = slice(512 * hl + 128 * qbl,
                                               512 * hl + 128 * (qbl + 1))
                                    nc.gpsimd.tensor_tensor(
                                        pt[:, bs], pt[:, bs],
                                        pats[:, 128 * pidx : 128 * (pidx + 1)],
                                        OP.mult)
                            pts[(g, ei)] = pt
                            if j == QT - 1:
                                emit_oproj(1)
                            if g == 1 and (ei + 1) % av_cadence == 0:
                                nqb = len([1 for x in av_done if x[0] == 0])
                                if nqb < 4:
                                    emit_av(0, nqb)
                    return run

                # split each g's entry stream into ~3 chunks for interleaving
                for g in range(NG):
                    eis = list(range(nent))
                    step = max(1, (nent + 3) // 4)
                    for s in range(0, nent, step):
                        chunks.append(score_chunk(g, eis[s : s + step]))

                def tail():
                    for g in range(NG):
                        for qbl in range(4):
                            if (g, qbl) not in av_done:
                                emit_av(g, qbl)
                    if j < QT - 1:
                        pending_oproj.extend(range(4 * j, 4 * j + 4))
                chunks.append(tail)
                return chunks

            # ---- pipelined emission ---------------------------------------
            prev_chunks = []

            def drain(n):
                for _ in range(n):
                    if prev_chunks:
                        prev_chunks.pop(0)()

            for nk in range(QT):
                dma_batch(nk)
                st_all = psA.tile([128, 512], f32, tag="A", name="st_all")
                nc.vector.memset(st_all[0:98], 1.0)
                ppcs = {}
                for g in range(NG):
                    for nm in ("wq", "wk"):
                        pp = proj_chain(nm, g, nk)
                        idx = 2 * g + (1 if nm == "wk" else 0)
                        ppcs[idx] = rms_stage1(nm, g, nk, pp, st_all)
                        drain(1)
                    rms_stage2(nk, g, st_all, ppcs)
                # flush the previous tile, then start this tile's score
                # stream before the v projections so the exp pipeline never
                # waits on them (v feeds only the later AV stage).
                drain(99)
                prev_chunks = attn_tile(nk)
                drain(2)
                for g in range(NG):
                    pp = proj_chain("wv", g, nk)
                    v_epilogue(g, nk, pp)
                    drain(1)
            drain(99)
            emit_oproj()

    nc.compile()
    return nc


def _pair_split(a):
    hi = a.astype(FP8)
    lo = (a - hi.astype(np.float32)).astype(FP8E5)
    return hi, lo


def _chunk_pair_x(xT):
    """[C, T] f32 -> two [128, C2*2*T] arrays (e4m3 body, e5m2 delta)."""
    hi, lo = _pair_split(xT)
    out = []
    for arr in (hi, lo):
        a = arr.reshape(C2, 2, 128, T).transpose(2, 0, 1, 3).reshape(128, -1)
        out.append(np.ascontiguousarray(a))
    return out


def _chunk_pair_w(W):
    """[C, 256] f32 -> two [128, C2*2*256] arrays."""
    hi, lo = _pair_split(W)
    out = []
    for arr in (hi, lo):
        a = arr.reshape(C2, 2, 128, 256).transpose(2, 0, 1, 3).reshape(128, -1)
        out.append(np.ascontiguousarray(a))
    return out


def kernel(**inputs):
    from concourse import bass_utils

    x = np.asarray(inputs["x"], np.float32)
    mask = np.asarray(inputs["attention_mask"])
    Wq = np.asarray(inputs["Wq"], np.float32)
    Wk = np.asarray(inputs["Wk"], np.float32)
    Wv = np.asarray(inputs["Wv"], np.float32)
    Wo = np.asarray(inputs["Wo"], np.float32)
    qw = np.asarray(inputs["q_norm_w"], np.float32)
    kw = np.asarray(inputs["k_norm_w"], np.float32)
    gate = np.asarray(inputs["gate"], np.float32).reshape(H)

    mask01 = mask.reshape(T, T) != 0
    plan, patterns = _analyze_mask(mask01)
    npat = patterns.shape[0]

    # fold the per-head gate into the value/output projections
    Wv = Wv / np.repeat(gate, D)[None, :]
    Wo = Wo * np.repeat(gate, D)[:, None]

    bound = 8.0 * np.max(np.abs(qw)) * np.max(np.abs(kw))
    neg_bias = -max(0.0, float(bound) - 60.0)

    key = (hash(mask01.tobytes()), npat, neg_bias)
    if key not in _CACHE:
        _CACHE[key] = _build_program(plan, npat, neg_bias)
    nc = _CACHE[key]

    pats_r = np.ascontiguousarray(
        patterns.transpose(1, 0, 2).reshape(128, 128 * npat)
    ).astype(BF16)
    sel2 = np.zeros((128, 2), np.float32)
    sel2[0:64, 0] = 1.0
    sel2[64:128, 1] = 1.0
    selT4 = np.zeros((34, 512), np.float32)
    for idx in range(4):
        w = qw if idx % 2 == 0 else kw
        r = 32 * (idx % 2)
        selT4[r + 0, 128 * idx : 128 * idx + 64] = w
        selT4[r + 1, 128 * idx + 64 : 128 * idx + 128] = w
    selT4 = selT4.astype(BF16)
    ident128 = np.eye(128, dtype=np.float32).astype(BF16)
    wq_col = np.tile(qw, 2)[:, None].astype(np.float32)
    wk_col = np.tile(kw, 2)[:, None].astype(np.float32)

    def chunk_major(W):
        ci, n = W.shape
        return np.ascontiguousarray(
            W.reshape(ci // 128, 128, n).transpose(1, 0, 2).reshape(128, -1)
        ).astype(BF16)

    in_maps = []
    for core in range(NCORES):
        b, grp = core // 4, core % 4
        hs = slice(256 * grp, 256 * (grp + 1))
        xT = np.ascontiguousarray(x[b].T)
        x8, xd = _chunk_pair_x(xT)
        m = {
            "x8": x8,
            "xd": xd,
            "wo": chunk_major(Wo[hs, :]),
            "wq_col": wq_col,
            "wk_col": wk_col,
            "sel2": sel2.astype(BF16),
            "selT4": selT4,
            "ident128": ident128,
            "pats": pats_r,
        }
        for nm, W in (("wq", Wq), ("wk", Wk), ("wv", Wv)):
            w8, wdl = _chunk_pair_w(W[:, hs])
            m[f"{nm}8"] = w8
            m[f"{nm}d"] = wdl
        in_maps.append(m)

    global _LAST_IN_MAPS
    _LAST_IN_MAPS = in_maps
    res = bass_utils.run_bass_kernel_spmd(nc, in_maps, core_ids=list(range(NCORES)))
    parts = [res.results[i]["out"].astype(np.float32) for i in range(NCORES)]
    out = np.stack(
        [
            parts[0] + parts[1] + parts[2] + parts[3],
            parts[4] + parts[5] + parts[6] + parts[7],
        ]
    )
    return out.astype(np.float32)


# revision 4
# speedup vs baseline: 1.0064x; 1.0064x over previous
"""Trainium2 Bass kernel for nn_Attention_20315195310831 (v2).

Fused attention block: q/k/v projections, per-head RMS-norm on q/k, masked
softmax with per-head gating, value residual, output projection.

Sharding over 8 NeuronCores: core = 4*b + grp handles batch b and heads
[4*grp, 4*grp+4). Each core computes its partial (attn_out + vx) @ Wo_slice;
the host sums the 4 partials per batch.

v2 changes vs the bf16 baseline:
- Projections run as fp8 DoubleRow matmuls (contraction 256/pass, 0.5
  cyc/col) with 3-term error compensation: x8@W8 + dx5@W8 + x8@dW5, where
  x8/W8 are e4m3 and the deltas are e5m2 (covers the small-residual range).
- Scores run as fp8 DoubleRow with the slot pair carrying (k8, k_lo)
  compensation levels at full D=64 contraction; q is single-quantized e4m3
  and its slot pair is a stride-0 broadcast_to view.
- Engine rebalance: Square/normalize drains on DVE (GPSIMD cannot touch
  PSUM), mask multiplies + fp8 k packing on Pool, AT transposes via
  SBUF->SBUF DMA transpose, softmax-denominator reciprocals batched.
- Software pipelining: projections for token-quarter nk are emitted
  interleaved with attention for q-tile j=nk-1, so the exp stream (the
  Activation-engine bottleneck) starts early and never starves.

PSUM (8 banks): S pool 2x[128,1024] (scores + AV accumulators, the AV tile
uses both its banks' independent zero-regions for the two head chains),
P pool 2x[128,512] (projection chains + output-projection accumulation),
A pool 2x (rms row-reduce / rms broadcast / v-transposes).
"""

import sys

sys.path.insert(0, "/opt/trn_rl_repo")

import ml_dtypes
import numpy as np

B, T, C = 2, 2048, 1024
H, D = 16, 64
EPS = 1e-5
SCALE = 1.0 / 8.0  # 1/sqrt(D)
NCORES = 8
HPC = 4  # heads per core
NG = 2  # head-pair groups per core
C2 = 4  # 256-row contraction chunks
QT = 4  # q tiles of 512
QW = 512
TBLK = T // 128
BF16 = ml_dtypes.bfloat16
FP8 = ml_dtypes.float8_e4m3
FP8E5 = ml_dtypes.float8_e5m2

_CACHE = {}
USE_DMA_TRANSPOSE = True


def _analyze_mask(mask01):
    """mask01: bool [T, T], mask01[q, k] True = attend.  (unchanged from v1)"""
    pat_index = {}
    patterns = []

    def pat_id(block_qk):
        add = np.where(block_qk.T, 1.0, 0.0).astype(np.float32)
        key = add.tobytes()
        if key not in pat_index:
            pat_index[key] = len(patterns)
            patterns.append(add)
        return pat_index[key]

    plan = []
    for j in range(QT):
        entries = []
        for kb in range(TBLK):
            qbs = []
            for qb in range(4):
                blk = mask01[
                    (4 * j + qb) * 128 : (4 * j + qb + 1) * 128,
                    kb * 128 : (kb + 1) * 128,
                ]
                qbs.append(blk)
            anyb = [b.any() for b in qbs]
            if not any(anyb):
                continue
            lo = anyb.index(True)
            hi = 4 - anyb[::-1].index(True)
            entries.append([kb, lo, hi, qbs])
        if entries:
            ulo = min(e[1] for e in entries)
            uhi = max(e[2] for e in entries)
            entries[0][1] = ulo
            entries[0][2] = uhi
        final = []
        for kb, lo, hi, qbs in entries:
            subs = []
            for qb in range(lo, hi):
                if not qbs[qb].all():
                    subs.append((qb, pat_id(qbs[qb])))
            final.append((kb, lo * 128, hi * 128, subs))
        plan.append(final)

    if not patterns:
        patterns.append(np.zeros((128, 128), np.float32))
    return plan, np.stack(patterns)


def _build_program(plan, npat, neg_bias):
    import concourse.mybir as mybir
    import concourse.tile as tile
    from concourse import bacc

    f32 = mybir.dt.float32
    bf16 = mybir.dt.bfloat16
    fp8 = mybir.dt.float8e4
    fp8e5 = mybir.dt.float8e5
    AF = mybir.ActivationFunctionType
    OP = mybir.AluOpType
    DR = mybir.MatmulPerfMode.DoubleRow

    nc = bacc.Bacc(
        "TRN2",
        target_bir_lowering=False,
        debug=False,
        enable_asserts=False,
        num_devices=NCORES,
    )

    # host layouts:
    #  x8/xd: [128, c2(4), slot(2), T]  (slot = K-row pair for DoubleRow)
    #  w8/wd: [128, c2(4), slot(2), 256hd]
    x8_d = nc.dram_tensor("x8", [128, C2 * 2 * T], fp8, kind="ExternalInput").ap()
    xd_d = nc.dram_tensor("xd", [128, C2 * 2 * T], fp8e5, kind="ExternalInput").ap()
    w8_d = {}
    wd_d = {}
    for nm in ("wq", "wk", "wv"):
        w8_d[nm] = nc.dram_tensor(f"{nm}8", [128, C2 * 2 * 256], fp8,
                                  kind="ExternalInput").ap()
        wd_d[nm] = nc.dram_tensor(f"{nm}d", [128, C2 * 2 * 256], fp8e5,
                                  kind="ExternalInput").ap()
    wo_d = nc.dram_tensor("wo", [128, 2048], bf16, kind="ExternalInput").ap()
    wqc_d = nc.dram_tensor("wq_col", [128, 1], f32, kind="ExternalInput").ap()
    wkc_d = nc.dram_tensor("wk_col", [128, 1], f32, kind="ExternalInput").ap()
    sel2_d = nc.dram_tensor("sel2", [128, 2], bf16, kind="ExternalInput").ap()
    selT4_d = nc.dram_tensor("selT4", [34, 512], bf16, kind="ExternalInput").ap()
    ident_d = nc.dram_tensor("ident128", [128, 128], bf16, kind="ExternalInput").ap()
    pats_d = nc.dram_tensor("pats", [128, 128 * npat], bf16, kind="ExternalInput").ap()
    out_d = nc.dram_tensor("out", [T, C], bf16, kind="ExternalOutput").ap()

    x8v = x8_d.rearrange("p (c s t) -> p c s t", c=C2, s=2)
    xdv = xd_d.rearrange("p (c s t) -> p c s t", c=C2, s=2)
    w8v = {nm: w8_d[nm].rearrange("p (c s h) -> p c s h", c=C2, s=2)
           for nm in w8_d}
    wdv = {nm: wd_d[nm].rearrange("p (c s h) -> p c s h", c=C2, s=2)
           for nm in wd_d}

    with tile.TileContext(nc) as tc, \
         nc.allow_low_precision(reason="fp8/bf16 staging validated against fp32 reference"):
        with tc.tile_pool(name="pers", bufs=1) as pers, \
             tc.tile_pool(name="rot", bufs=4) as rot, \
             tc.tile_pool(name="ptp", bufs=34) as ptp, \
             tc.tile_pool(name="obp", bufs=4) as obp, \
             tc.tile_pool(name="psS", bufs=2, space="PSUM") as psS, \
             tc.tile_pool(name="psP", bufs=2, space="PSUM") as psP, \
             tc.tile_pool(name="psA", bufs=2, space="PSUM") as psA:

            # ---- persistent tiles
            x8t = [pers.tile([128, 2, T], fp8, tag=f"x8_{c}", name=f"x8_{c}")
                   for c in range(C2)]
            xdt = [pers.tile([128, 2, T], fp8e5, tag=f"xd_{c}", name=f"xd_{c}")
                   for c in range(C2)]
            w8 = {nm: pers.tile([128, C2, 2, 256], fp8, tag=f"{nm}8sb",
                                name=f"{nm}8sb") for nm in ("wq", "wk", "wv")}
            wd = {nm: pers.tile([128, C2, 2, 256], fp8e5, tag=f"{nm}dsb",
                                name=f"{nm}dsb") for nm in ("wq", "wk", "wv")}
            wo_sb = pers.tile([128, 2048], bf16, tag="wo_sb", name="wo_sb")
            q8T = [pers.tile([128, T], fp8, tag=f"q8T{g}", name=f"q8T{g}")
                   for g in range(NG)]
            kDR = [pers.tile([128, 2, T], fp8, tag=f"kDR{g}", name=f"kDR{g}")
                   for g in range(NG)]
            vT = [pers.tile([128, T], bf16, tag=f"vT{g}", name=f"vT{g}")
                  for g in range(NG)]
            vaug = [pers.tile([128, 65 * HPC], bf16, tag=f"vaug{kb}",
                              name=f"vaug{kb}") for kb in range(TBLK)]
            AT = [pers.tile([128, T], bf16, tag=f"AT{g}", name=f"AT{g}")
                  for g in range(NG)]
            AT_q = [pers.tile([128, 256], bf16, tag=f"ATq{qbl}", name=f"ATq{qbl}")
                    for qbl in range(4)]
            wq_col = pers.tile([128, 1], f32, tag="wq_col_sb", name="wq_col_sb")
            wk_col = pers.tile([128, 1], f32, tag="wk_col_sb", name="wk_col_sb")
            sel2 = pers.tile([128, 2], bf16, tag="sel2_sb", name="sel2_sb")
            selT4 = pers.tile([34, 512], bf16, tag="selT4_sb", name="selT4_sb")
            ident = pers.tile([128, 128], bf16, tag="ident_sb", name="ident_sb")
            pats = pers.tile([128, 128 * npat], bf16, tag="pats_sb", name="pats_sb")
            eps_col = pers.tile([128, 1], f32, tag="eps_col", name="eps_col")
            nb_col = pers.tile([128, 1], f32, tag="nb_col", name="nb_col")
            one_col = pers.tile([128, 1], bf16, tag="one_col", name="one_col")
            nc.vector.memset(eps_col, EPS)
            nc.vector.memset(nb_col, neg_bias)
            nc.vector.memset(one_col, 1.0)

            for kb in range(TBLK):
                for h in range(HPC):
                    nc.gpsimd.tensor_copy(vaug[kb][:, 65 * h + 64 : 65 * h + 65],
                                          one_col)

            # ---- DMA staging plan -----------------------------------------
            def dma_batch(nk):
                if nk == 0:
                    nc.sync.dma_start(sel2, sel2_d)
                    nc.sync.dma_start(selT4, selT4_d)
                    nc.sync.dma_start(wq_col, wqc_d)
                    nc.sync.dma_start(wk_col, wkc_d)
                    nc.sync.dma_start(w8["wq"].rearrange("p a b c -> p (a b c)"), w8_d["wq"])
                    nc.sync.dma_start(w8["wk"].rearrange("p a b c -> p (a b c)"), w8_d["wk"])
                    for c in range(C2):
                        nc.sync.dma_start(x8t[c][:, :, 0:512], x8v[:, c, :, 0:512])
                    nc.sync.dma_start(wd["wq"].rearrange("p a b c -> p (a b c)"), wd_d["wq"])
                    nc.sync.dma_start(wd["wk"].rearrange("p a b c -> p (a b c)"), wd_d["wk"])
                    for c in range(C2):
                        nc.sync.dma_start(xdt[c][:, :, 0:512], xdv[:, c, :, 0:512])
                    nc.sync.dma_start(w8["wv"].rearrange("p a b c -> p (a b c)"), w8_d["wv"])
                    nc.sync.dma_start(wd["wv"].rearrange("p a b c -> p (a b c)"), wd_d["wv"])
                    nc.sync.dma_start(ident, ident_d)
                    nc.sync.dma_start(pats, pats_d)
                else:
                    cs = slice(512 * nk, 512 * (nk + 1))
                    for c in range(C2):
                        nc.sync.dma_start(x8t[c][:, :, cs], x8v[:, c, :, cs])
                    for c in range(C2):
                        nc.sync.dma_start(xdt[c][:, :, cs], xdv[:, c, :, cs])
                    if nk == 1:
                        nc.sync.dma_start(wo_sb, wo_d)

            # ---- projection chain for one (nm, g, nk) ---------------------
            def proj_chain(nm, g, nk):
                cs = slice(512 * nk, 512 * (nk + 1))
                hs = slice(128 * g, 128 * (g + 1))
                pp = psP.tile([128, 512], f32, tag="P", name=f"pp_{nm}{g}{nk}")
                n = 0
                for c in range(C2):
                    nc.tensor.matmul(pp, w8[nm][:, c, :, hs], x8t[c][:, :, cs],
                                     start=(n == 0), stop=False, perf_mode=DR)
                    n += 1
                for c in range(C2):
                    nc.tensor.matmul(pp, w8[nm][:, c, :, hs], xdt[c][:, :, cs],
                                     start=False, stop=False, perf_mode=DR)
                    n += 1
                for c in range(C2):
                    nc.tensor.matmul(pp, wd[nm][:, c, :, hs], x8t[c][:, :, cs],
                                     start=False, stop=(c == C2 - 1), perf_mode=DR)
                    n += 1
                return pp

            def rms_stage1(nm, g, nk, pp, st_all):
                """square (Pool) + row-reduce into the shared st tile.

                pp is released after the bf16 copy; the copy feeds both the
                square and the later normalize stt (keeps stt off PSUM)."""
                idx = 2 * g + (1 if nm == "wk" else 0)
                ppc = rot.tile([128, 512], bf16, tag=f"ppc{idx}",
                               name="ppc", bufs=2)
                nc.vector.tensor_copy(ppc, pp)
                sq = rot.tile([128, 512], bf16, tag="sq", name="sq", bufs=4)
                nc.gpsimd.tensor_tensor(sq, ppc, ppc, OP.mult)
                nc.tensor.matmul(st_all[32 * idx : 32 * idx + 2], sel2, sq,
                                 start=True, stop=True,
                                 tile_position=(0, 32 * idx))
                return ppc

            def rms_stage2(nk, g, st_all, ppcs):
                """one ln+exp over this group's (q,k) rms rows, then per-chain
                broadcast + fp8 normalize (Pool, all-SBUF).

                1/rms = exp(-0.5*ln(ms/D + eps)): ln and exp share the exp
                activation table, so the attention exp stream never reloads
                ACT tables (sqrt would force a reload per call)."""
                cs = slice(512 * nk, 512 * (nk + 1))
                base = 64 * g
                lnr = rot.tile([34, 512], bf16, tag="lnr", name="lnr", bufs=2)
                nc.scalar.activation(lnr, st_all[base : base + 34], AF.Ln,
                                     bias=eps_col[0:34], scale=1.0 / D)
                rinv = rot.tile([34, 512], bf16, tag="rinv", name="rinv",
                                bufs=2)
                nc.scalar.activation(rinv, lnr, AF.Exp, scale=-0.5)
                for nm in ("wq", "wk"):
                    idx = 2 * g + (1 if nm == "wk" else 0)
                    mm = psA.tile([128, 512], f32, tag="A", name="mm_ps")
                    nc.tensor.matmul(
                        mm, selT4[0:34, 128 * idx : 128 * (idx + 1)],
                        rinv, start=True, stop=True)
                    mrbc = rot.tile([128, 512], bf16, tag="mrbc",
                                    name="mrbc", bufs=2)
                    nc.vector.tensor_copy(mrbc, mm)
                    ppc = ppcs[idx]
                    if nm == "wq":
                        nc.gpsimd.tensor_tensor(
                            q8T[g][:, cs], ppc, mrbc, OP.mult)
                    else:
                        ktmp = rot.tile([128, 512], bf16, tag="ktmp",
                                        name="ktmp", bufs=2)
                        nc.gpsimd.tensor_tensor(
                            ktmp, ppc, mrbc, OP.mult)
                        nc.gpsimd.tensor_copy(kDR[g][:, 0, cs], ktmp)
                        nc.gpsimd.tensor_tensor(kDR[g][:, 1, cs], ktmp,
                                                kDR[g][:, 0, cs],
                                                OP.subtract)

            def v_epilogue(g, nk, pp):
                cs = slice(512 * nk, 512 * (nk + 1))
                nc.vector.tensor_copy(vT[g][:, cs], pp)
                for kb in range(4 * nk, 4 * nk + 4):
                    pv = psA.tile([128, 128], bf16, tag="A", name="vtr_ps")
                    nc.tensor.transpose(pv, vT[g][:, 128 * kb : 128 * (kb + 1)],
                                        ident)
                    dst = vaug[kb].rearrange("p (h c) -> p h c", h=HPC)[
                        :, 2 * g : 2 * g + 2, 0:64]
                    nc.vector.tensor_copy(
                        dst, pv.rearrange("p (h c) -> p h c", h=2))

            # ---- attention machinery --------------------------------------
            pending_oproj = []

            def emit_oproj(count=99):
                while pending_oproj and count > 0:
                    count -= 1
                    tb = pending_oproj.pop(0)
                    for nn in range(2):
                        po = psP.tile([128, 512], f32, tag="P", name="po_ps")
                        for cg in range(NG):
                            nc.tensor.matmul(
                                po,
                                AT[cg][:, 128 * tb : 128 * (tb + 1)],
                                wo_sb[:, 1024 * cg + 512 * nn :
                                      1024 * cg + 512 * (nn + 1)],
                                start=(cg == 0),
                                stop=(cg == NG - 1),
                            )
                        ob = obp.tile([128, 512], bf16, tag="ob", name="ob")
                        nc.vector.tensor_copy(ob, po)
                        nc.sync.dma_start(
                            out_d[128 * tb : 128 * (tb + 1),
                                  512 * nn : 512 * (nn + 1)],
                            ob,
                        )

            def attn_tile(j):
                """emit attention for q-tile j as a list of chunks; caller
                interleaves the chunks with projection chains."""
                entries = plan[j]
                if not entries:
                    return []
                nent = len(entries)
                last_for_qb = {}
                for ei, (kb, ql, qh, subs) in enumerate(entries):
                    for qbl in range(ql // 128, qh // 128):
                        last_for_qb[qbl] = ei
                pts = {}
                av_done = set()

                def emit_av(g, qbl):
                    av_done.add((g, qbl))
                    o_q = [psP.tile([128, 512], f32, tag="P", name="o_q")
                           for _ in range(2)]
                    started = False
                    for ei, (kb, ql, qh, subs) in enumerate(entries):
                        if not (ql <= 128 * qbl < qh):
                            continue
                        ptd, delta = pts[(g, ei)]
                        for hl in range(2):
                            h = 2 * g + hl
                            nc.tensor.matmul(
                                o_q[hl][:, 0:65],
                                ptd[:, 512 * hl + delta + 128 * qbl :
                                    512 * hl + delta + 128 * (qbl + 1)],
                                vaug[kb][:, 65 * h : 65 * (h + 1)],
                                start=not started,
                                stop=(ei == last_for_qb[qbl]),
                            )
                        started = True
                    minv2 = rot.tile([128, 2], f32, tag="mv", name="minv2",
                                     bufs=4)
                    for hl in range(2):
                        nc.vector.reciprocal(minv2[:, hl : hl + 1],
                                             o_q[hl][:, 64:65])
                    for hl in range(2):
                        nc.vector.scalar_tensor_tensor(
                            AT_q[qbl][:, 128 * g + 64 * hl :
                                      128 * g + 64 * (hl + 1)],
                            o_q[hl][:, 0:64],
                            minv2[:, hl : hl + 1],
                            vaug[4 * j + qbl][
                                :, 65 * (2 * g + hl) : 65 * (2 * g + hl) + 64],
                            OP.mult,
                            OP.add,
                        )
                    if j == QT - 1:
                        emit_oproj(1)
                    if g == NG - 1:
                        tb = 4 * j + qbl
                        if USE_DMA_TRANSPOSE and j < QT - 1:
                            for cg in range(NG):
                                nc.sync.dma_start_transpose(
                                    AT[cg][:, 128 * tb : 128 * (tb + 1)],
                                    AT_q[qbl][:, 128 * cg : 128 * (cg + 1)],
                                )
                        else:
                            for cg in range(NG):
                                tps = psA.tile([128, 128], bf16, tag="A",
                                               name="tps")
                                nc.tensor.transpose(
                                    tps, AT_q[qbl][:, 128 * cg : 128 * (cg + 1)],
                                    ident)
                                nc.vector.tensor_copy(
                                    AT[cg][:, 128 * tb : 128 * (tb + 1)], tps)
                        if j == QT - 1:
                            pending_oproj.append(tb)
                            emit_oproj(1)

                chunks = []
                av_cadence = max(1, nent // 4)

                def score_chunk(g, eis):
                    def run():
                        # pack consecutive taper entries (w1+w2 <= 512) into
                        # one S tile / one exp call
                        groups = []
                        i = 0
                        while i < len(eis):
                            ei = eis[i]
                            if i + 1 < len(eis):
                                e2 = eis[i + 1]
                                w1 = entries[ei][2] - entries[ei][1]
                                w2 = entries[e2][2] - entries[e2][1]
                                if w1 + w2 <= 512:
                                    groups.append([(ei, 0), (e2, w1)])
                                    i += 2
                                    continue
                            groups.append([(ei, None)])
                            i += 1
                        for group in groups:
                            s_ps = psS.tile([128, 1024], f32, tag="S",
                                            name="s_ps")
                            pt = ptp.tile([128, 1024], bf16, tag="PT",
                                          name="pt")
                            s3 = s_ps.rearrange("p (h w) -> p h w", h=2)
                            p3 = pt.rearrange("p (h w) -> p h w", h=2)
                            lo, hi = 512, 0
                            for ei, base in group:
                                kb, ql, qh, subs = entries[ei]
                                w = qh - ql
                                b = ql if base is None else base
                                lo, hi = min(lo, b), max(hi, b + w)
                                for hl in range(2):
                                    nc.tensor.matmul(
                                        s_ps[:, 512 * hl + b : 512 * hl + b + w],
                                        kDR[g][64 * hl : 64 * (hl + 1), :,
                                               128 * kb : 128 * (kb + 1)],
                                        q8T[g][64 * hl : 64 * (hl + 1),
                                               512 * j + ql : 512 * j + qh]
                                        .unsqueeze(1).broadcast_to([64, 2, w]),
                                        start=True,
                                        stop=True,
                                        perf_mode=DR,
                                        tile_position=(64 * hl, 0),
                                    )
                            nc.scalar.activation(
                                p3[:, :, lo:hi], s3[:, :, lo:hi], AF.Exp,
                                bias=nb_col, scale=SCALE)
                            for ei, base in group:
                                kb, ql, qh, subs = entries[ei]
                                b = ql if base is None else base
                                delta = b - ql
                                for hl in range(2):
                                    for qbl, pidx in subs:
                                        bs = slice(
                                            512 * hl + delta + 128 * qbl,
                                            512 * hl + delta + 128 * (qbl + 1))
                                        nc.gpsimd.tensor_tensor(
                                            pt[:, bs], pt[:, bs],
                                            pats[:, 128 * pidx :
                                                 128 * (pidx + 1)],
                                            OP.mult)
                                pts[(g, ei)] = (pt, delta)
                            ei = group[-1][0]
                            if j == QT - 1:
                                emit_oproj(1)
                            if g == 1 and (ei + 1) % av_cadence == 0:
                                nqb = len([1 for x in av_done if x[0] == 0])
                                if nqb < 4:
                                    emit_av(0, nqb)
                    return run

                # split each g's entry stream into ~3 chunks for interleaving
                for g in range(NG):
                    eis = list(range(nent))
                    step = max(1, (nent + 3) // 4)
                    for s in range(0, nent, step):
                        chunks.append(score_chunk(g, eis[s : s + step]))

                def tail():
                    for g in range(NG):
                        for qbl in range(4):
                            if (g, qbl) not in av_done:
                                emit_av(g, qbl)
                    if j < QT - 1:
                        pending_oproj.extend(range(4 * j, 4 * j + 4))
                chunks.append(tail)
                return chunks

            # ---- pipelined emission ---------------------------------------
            prev_chunks = []

            def drain(n):
                for _ in range(n):
                    if prev_chunks:
                        prev_chunks.pop(0)()

            for nk in range(QT):
                dma_batch(nk)
                st_all = psA.tile([128, 512], f32, tag="A", name="st_all")
                nc.vector.memset(st_all[0:98], 1.0)
                ppcs = {}
                for g in range(NG):
                    for nm in ("wq", "wk"):
                        pp = proj_chain(nm, g, nk)
                        idx = 2 * g + (1 if nm == "wk" else 0)
                        ppcs[idx] = rms_stage1(nm, g, nk, pp, st_all)
                        drain(1)
                    rms_stage2(nk, g, st_all, ppcs)
                # flush the previous tile, then start this tile's score
                # stream before the v projections so the exp pipeline never
                # waits on them (v feeds only the later AV stage).
                drain(99)
                prev_chunks = attn_tile(nk)
                drain(2)
                for g in range(NG):
                    pp = proj_chain("wv", g, nk)
                    v_epilogue(g, nk, pp)
                    drain(1)
            drain(99)
            emit_oproj()

    nc.compile()
    return nc


def _pair_split(a):
    hi = a.astype(FP8)
    lo = (a - hi.astype(np.float32)).astype(FP8E5)
    return hi, lo


def _chunk_pair_x(xT):
    """[C, T] f32 -> two [128, C2*2*T] arrays (e4m3 body, e5m2 delta)."""
    hi, lo = _pair_split(xT)
    out = []
    for arr in (hi, lo):
        a = arr.reshape(C2, 2, 128, T).transpose(2, 0, 1, 3).reshape(128, -1)
        out.append(np.ascontiguousarray(a))
    return out


def _chunk_pair_w(W):
    """[C, 256] f32 -> two [128, C2*2*256] arrays."""
    hi, lo = _pair_split(W)
    out = []
    for arr in (hi, lo):
        a = arr.reshape(C2, 2, 128, 256).transpose(2, 0, 1, 3).reshape(128, -1)
        out.append(np.ascontiguousarray(a))
    return out


def kernel(**inputs):
    from concourse import bass_utils

    x = np.asarray(inputs["x"], np.float32)
    mask = np.asarray(inputs["attention_mask"])
    Wq = np.asarray(inputs["Wq"], np.float32)
    Wk = np.asarray(inputs["Wk"], np.float32)
    Wv = np.asarray(inputs["Wv"], np.float32)
    Wo = np.asarray(inputs["Wo"], np.float32)
    qw = np.asarray(inputs["q_norm_w"], np.float32)
    kw = np.asarray(inputs["k_norm_w"], np.float32)
    gate = np.asarray(inputs["gate"], np.float32).reshape(H)

    mask01 = mask.reshape(T, T) != 0
    plan, patterns = _analyze_mask(mask01)
    npat = patterns.shape[0]

    # fold the per-head gate into the value/output projections
    Wv = Wv / np.repeat(gate, D)[None, :]
    Wo = Wo * np.repeat(gate, D)[:, None]

    bound = 8.0 * np.max(np.abs(qw)) * np.max(np.abs(kw))
    neg_bias = -max(0.0, float(bound) - 60.0)

    key = (hash(mask01.tobytes()), npat, neg_bias)
    if key not in _CACHE:
        _CACHE[key] = _build_program(plan, npat, neg_bias)
    nc = _CACHE[key]

    pats_r = np.ascontiguousarray(
        patterns.transpose(1, 0, 2).reshape(128, 128 * npat)
    ).astype(BF16)
    sel2 = np.zeros((128, 2), np.float32)
    sel2[0:64, 0] = 1.0
    sel2[64:128, 1] = 1.0
    selT4 = np.zeros((34, 512), np.float32)
    for idx in range(4):
        w = qw if idx % 2 == 0 else kw
        r = 32 * (idx % 2)
        selT4[r + 0, 128 * idx : 128 * idx + 64] = w
        selT4[r + 1, 128 * idx + 64 : 128 * idx + 128] = w
    selT4 = selT4.astype(BF16)
    ident128 = np.eye(128, dtype=np.float32).astype(BF16)
    wq_col = np.tile(qw, 2)[:, None].astype(np.float32)
    wk_col = np.tile(kw, 2)[:, None].astype(np.float32)

    def chunk_major(W):
        ci, n = W.shape
        return np.ascontiguousarray(
            W.reshape(ci // 128, 128, n).transpose(1, 0, 2).reshape(128, -1)
        ).astype(BF16)

    in_maps = []
    for core in range(NCORES):
        b, grp = core // 4, core % 4
        hs = slice(256 * grp, 256 * (grp + 1))
        xT = np.ascontiguousarray(x[b].T)
        x8, xd = _chunk_pair_x(xT)
        m = {
            "x8": x8,
            "xd": xd,
            "wo": chunk_major(Wo[hs, :]),
            "wq_col": wq_col,
            "wk_col": wk_col,
            "sel2": sel2.astype(BF16),
            "selT4": selT4,
            "ident128": ident128,
            "pats": pats_r,
        }
        for nm, W in (("wq", Wq), ("wk", Wk), ("wv", Wv)):
            w8, wdl = _chunk_pair_w(W[:, hs])
            m[f"{nm}8"] = w8
            m[f"{nm}d"] = wdl
        in_maps.append(m)

    global _LAST_IN_MAPS
    _LAST_IN_MAPS = in_maps
    res = bass_utils.run_bass_kernel_spmd(nc, in_maps, core_ids=list(range(NCORES)))
    parts = [res.results[i]["out"].astype(np.float32) for i in range(NCORES)]
    out = np.stack(
        [
            parts[0] + parts[1] + parts[2] + parts[3],
            parts[4] + parts[5] + parts[6] + parts[7],
        ]
    )
    return out.astype(np.float32)


# revision 6
# speedup vs baseline: 1.0131x; 1.0066x over previous
"""Trainium2 Bass kernel for nn_Attention_20315195310831 (v2).

Fused attention block: q/k/v projections, per-head RMS-norm on q/k, masked
softmax with per-head gating, value residual, output projection.

Sharding over 8 NeuronCores: core = 4*b + grp handles batch b and heads
[4*grp, 4*grp+4). Each core computes its partial (attn_out + vx) @ Wo_slice;
the host sums the 4 partials per batch.

v2 changes vs the bf16 baseline:
- Projections run as fp8 DoubleRow matmuls (contraction 256/pass, 0.5
  cyc/col) with 3-term error compensation: x8@W8 + dx5@W8 + x8@dW5, where
  x8/W8 are e4m3 and the deltas are e5m2 (covers the small-residual range).
- Scores run as fp8 DoubleRow with the slot pair carrying (k8, k_lo)
  compensation levels at full D=64 contraction; q is single-quantized e4m3
  and its slot pair is a stride-0 broadcast_to view.
- Engine rebalance: Square/normalize drains on DVE (GPSIMD cannot touch
  PSUM), mask multiplies + fp8 k packing on Pool, AT transposes via
  SBUF->SBUF DMA transpose, softmax-denominator reciprocals batched.
- Software pipelining: projections for token-quarter nk are emitted
  interleaved with attention for q-tile j=nk-1, so the exp stream (the
  Activation-engine bottleneck) starts early and never starves.

PSUM (8 banks): S pool 2x[128,1024] (scores + AV accumulators, the AV tile
uses both its banks' independent zero-regions for the two head chains),
P pool 2x[128,512] (projection chains + output-projection accumulation),
A pool 2x (rms row-reduce / rms broadcast / v-transposes).
"""

import sys

sys.path.insert(0, "/opt/trn_rl_repo")

import ml_dtypes
import numpy as np

B, T, C = 2, 2048, 1024
H, D = 16, 64
EPS = 1e-5
SCALE = 1.0 / 8.0  # 1/sqrt(D)
NCORES = 8
HPC = 4  # heads per core
NG = 2  # head-pair groups per core
C2 = 4  # 256-row contraction chunks
QT = 4  # q tiles of 512
QW = 512
TBLK = T // 128
BF16 = ml_dtypes.bfloat16
FP8 = ml_dtypes.float8_e4m3
FP8E5 = ml_dtypes.float8_e5m2

_CACHE = {}
USE_DMA_TRANSPOSE = True


def _analyze_mask(mask01):
    """mask01: bool [T, T], mask01[q, k] True = attend.  (unchanged from v1)"""
    pat_index = {}
    patterns = []

    def pat_id(block_qk):
        add = np.where(block_qk.T, 1.0, 0.0).astype(np.float32)
        key = add.tobytes()
        if key not in pat_index:
            pat_index[key] = len(patterns)
            patterns.append(add)
        return pat_index[key]

    plan = []
    for j in range(QT):
        entries = []
        for kb in range(TBLK):
            qbs = []
            for qb in range(4):
                blk = mask01[
                    (4 * j + qb) * 128 : (4 * j + qb + 1) * 128,
                    kb * 128 : (kb + 1) * 128,
                ]
                qbs.append(blk)
            anyb = [b.any() for b in qbs]
            if not any(anyb):
                continue
            lo = anyb.index(True)
            hi = 4 - anyb[::-1].index(True)
            entries.append([kb, lo, hi, qbs])
        if entries:
            ulo = min(e[1] for e in entries)
            uhi = max(e[2] for e in entries)
            entries[0][1] = ulo
            entries[0][2] = uhi
        final = []
        for kb, lo, hi, qbs in entries:
            subs = []
            for qb in range(lo, hi):
                if not qbs[qb].all():
                    subs.append((qb, pat_id(qbs[qb])))
            final.append((kb, lo * 128, hi * 128, subs))
        plan.append(final)

    if not patterns:
        patterns.append(np.zeros((128, 128), np.float32))
    return plan, np.stack(patterns)


def _build_program(plan, npat, neg_bias):
    import concourse.mybir as mybir
    import concourse.tile as tile
    from concourse import bacc

    f32 = mybir.dt.float32
    bf16 = mybir.dt.bfloat16
    fp8 = mybir.dt.float8e4
    fp8e5 = mybir.dt.float8e5
    AF = mybir.ActivationFunctionType
    OP = mybir.AluOpType
    DR = mybir.MatmulPerfMode.DoubleRow

    nc = bacc.Bacc(
        "TRN2",
        target_bir_lowering=False,
        debug=False,
        enable_asserts=False,
        num_devices=NCORES,
    )

    # host layouts:
    #  x8/xd: [128, c2(4), slot(2), T]  (slot = K-row pair for DoubleRow)
    #  w8/wd: [128, c2(4), slot(2), 256hd]
    x8_d = nc.dram_tensor("x8", [128, C2 * 2 * T], fp8, kind="ExternalInput").ap()
    xd_d = nc.dram_tensor("xd", [128, C2 * 2 * T], fp8e5, kind="ExternalInput").ap()
    w8_d = {}
    wd_d = {}
    for nm in ("wq", "wk", "wv"):
        w8_d[nm] = nc.dram_tensor(f"{nm}8", [128, C2 * 2 * 256], fp8,
                                  kind="ExternalInput").ap()
        wd_d[nm] = nc.dram_tensor(f"{nm}d", [128, C2 * 2 * 256], fp8e5,
                                  kind="ExternalInput").ap()
    wo_d = nc.dram_tensor("wo", [128, 2048], bf16, kind="ExternalInput").ap()
    wqc_d = nc.dram_tensor("wq_col", [128, 1], f32, kind="ExternalInput").ap()
    wkc_d = nc.dram_tensor("wk_col", [128, 1], f32, kind="ExternalInput").ap()
    sel2_d = nc.dram_tensor("sel2", [128, 2], bf16, kind="ExternalInput").ap()
    selT4_d = nc.dram_tensor("selT4", [34, 512], bf16, kind="ExternalInput").ap()
    ident_d = nc.dram_tensor("ident128", [128, 128], bf16, kind="ExternalInput").ap()
    pats_d = nc.dram_tensor("pats", [128, 128 * npat], bf16, kind="ExternalInput").ap()
    out_d = nc.dram_tensor("out", [T, C], bf16, kind="ExternalOutput").ap()

    x8v = x8_d.rearrange("p (c s t) -> p c s t", c=C2, s=2)
    xdv = xd_d.rearrange("p (c s t) -> p c s t", c=C2, s=2)
    w8v = {nm: w8_d[nm].rearrange("p (c s h) -> p c s h", c=C2, s=2)
           for nm in w8_d}
    wdv = {nm: wd_d[nm].rearrange("p (c s h) -> p c s h", c=C2, s=2)
           for nm in wd_d}

    with tile.TileContext(nc) as tc, \
         nc.allow_low_precision(reason="fp8/bf16 staging validated against fp32 reference"):
        with tc.tile_pool(name="pers", bufs=1) as pers, \
             tc.tile_pool(name="rot", bufs=4) as rot, \
             tc.tile_pool(name="ptp", bufs=34) as ptp, \
             tc.tile_pool(name="obp", bufs=4) as obp, \
             tc.tile_pool(name="psS", bufs=2, space="PSUM") as psS, \
             tc.tile_pool(name="psP", bufs=2, space="PSUM") as psP, \
             tc.tile_pool(name="psA", bufs=2, space="PSUM") as psA:

            # ---- persistent tiles
            x8t = [pers.tile([128, 2, T], fp8, tag=f"x8_{c}", name=f"x8_{c}")
                   for c in range(C2)]
            xdt = [pers.tile([128, 2, T], fp8e5, tag=f"xd_{c}", name=f"xd_{c}")
                   for c in range(C2)]
            w8 = {nm: pers.tile([128, C2, 2, 256], fp8, tag=f"{nm}8sb",
                                name=f"{nm}8sb") for nm in ("wq", "wk", "wv")}
            wd = {nm: pers.tile([128, C2, 2, 256], fp8e5, tag=f"{nm}dsb",
                                name=f"{nm}dsb") for nm in ("wq", "wk", "wv")}
            wo_sb = pers.tile([128, 2048], bf16, tag="wo_sb", name="wo_sb")
            q8T = [pers.tile([128, T], fp8, tag=f"q8T{g}", name=f"q8T{g}")
                   for g in range(NG)]
            kDR = [pers.tile([128, 2, T], fp8, tag=f"kDR{g}", name=f"kDR{g}")
                   for g in range(NG)]
            vT = [pers.tile([128, T], bf16, tag=f"vT{g}", name=f"vT{g}")
                  for g in range(NG)]
            vaug = [pers.tile([128, 65 * HPC], bf16, tag=f"vaug{kb}",
                              name=f"vaug{kb}") for kb in range(TBLK)]
            AT = [pers.tile([128, T], bf16, tag=f"AT{g}", name=f"AT{g}")
                  for g in range(NG)]
            AT_q = [pers.tile([128, 256], bf16, tag=f"ATq{qbl}", name=f"ATq{qbl}")
                    for qbl in range(4)]
            wq_col = pers.tile([128, 1], f32, tag="wq_col_sb", name="wq_col_sb")
            wk_col = pers.tile([128, 1], f32, tag="wk_col_sb", name="wk_col_sb")
            sel2 = pers.tile([128, 2], bf16, tag="sel2_sb", name="sel2_sb")
            selT4 = pers.tile([34, 512], bf16, tag="selT4_sb", name="selT4_sb")
            ident = pers.tile([128, 128], bf16, tag="ident_sb", name="ident_sb")
            pats = pers.tile([128, 128 * npat], bf16, tag="pats_sb", name="pats_sb")
            eps_col = pers.tile([128, 1], f32, tag="eps_col", name="eps_col")
            nb_col = pers.tile([128, 1], f32, tag="nb_col", name="nb_col")
            one_col = pers.tile([128, 1], bf16, tag="one_col", name="one_col")
            nc.vector.memset(eps_col, EPS)
            nc.vector.memset(nb_col, neg_bias)
            nc.vector.memset(one_col, 1.0)

            for kb in range(TBLK):
                for h in range(HPC):
                    nc.gpsimd.tensor_copy(vaug[kb][:, 65 * h + 64 : 65 * h + 65],
                                          one_col)

            # ---- DMA staging plan -----------------------------------------
            def dma_batch(nk):
                if nk == 0:
                    nc.sync.dma_start(sel2, sel2_d)
                    nc.sync.dma_start(selT4, selT4_d)
                    nc.sync.dma_start(wq_col, wqc_d)
                    nc.sync.dma_start(wk_col, wkc_d)
                    nc.sync.dma_start(w8["wq"].rearrange("p a b c -> p (a b c)"), w8_d["wq"])
                    nc.sync.dma_start(w8["wk"].rearrange("p a b c -> p (a b c)"), w8_d["wk"])
                    for c in range(C2):
                        nc.sync.dma_start(x8t[c][:, :, 0:512], x8v[:, c, :, 0:512])
                    nc.sync.dma_start(wd["wq"].rearrange("p a b c -> p (a b c)"), wd_d["wq"])
                    nc.sync.dma_start(wd["wk"].rearrange("p a b c -> p (a b c)"), wd_d["wk"])
                    for c in range(C2):
                        nc.sync.dma_start(xdt[c][:, :, 0:512], xdv[:, c, :, 0:512])
                    nc.sync.dma_start(w8["wv"].rearrange("p a b c -> p (a b c)"), w8_d["wv"])
                    nc.sync.dma_start(wd["wv"].rearrange("p a b c -> p (a b c)"), wd_d["wv"])
                    nc.sync.dma_start(ident, ident_d)
                    nc.sync.dma_start(pats, pats_d)
                else:
                    cs = slice(512 * nk, 512 * (nk + 1))
                    for c in range(C2):
                        nc.sync.dma_start(x8t[c][:, :, cs], x8v[:, c, :, cs])
                    for c in range(C2):
                        nc.sync.dma_start(xdt[c][:, :, cs], xdv[:, c, :, cs])
                    if nk == 1:
                        nc.sync.dma_start(wo_sb, wo_d)

            # ---- projection chain for one (nm, g, nk) ---------------------
            def proj_chain(nm, g, nk):
                cs = slice(512 * nk, 512 * (nk + 1))
                hs = slice(128 * g, 128 * (g + 1))
                pp = psP.tile([128, 512], f32, tag="P", name=f"pp_{nm}{g}{nk}")
                n = 0
                for c in range(C2):
                    nc.tensor.matmul(pp, w8[nm][:, c, :, hs], x8t[c][:, :, cs],
                                     start=(n == 0), stop=False, perf_mode=DR)
                    n += 1
                for c in range(C2):
                    nc.tensor.matmul(pp, w8[nm][:, c, :, hs], xdt[c][:, :, cs],
                                     start=False, stop=False, perf_mode=DR)
                    n += 1
                for c in range(C2):
                    nc.tensor.matmul(pp, wd[nm][:, c, :, hs], x8t[c][:, :, cs],
                                     start=False, stop=(c == C2 - 1), perf_mode=DR)
                    n += 1
                return pp

            def rms_stage1(nm, g, nk, pp, st_all):
                """square (Pool) + row-reduce into the shared st tile.

                pp is released after the bf16 copy; the copy feeds both the
                square and the later normalize stt (keeps stt off PSUM)."""
                idx = 2 * g + (1 if nm == "wk" else 0)
                ppc = rot.tile([128, 512], bf16, tag=f"ppc{idx}",
                               name="ppc", bufs=2)
                nc.vector.tensor_copy(ppc, pp)
                sq = rot.tile([128, 512], bf16, tag="sq", name="sq", bufs=4)
                nc.gpsimd.tensor_tensor(sq, ppc, ppc, OP.mult)
                nc.tensor.matmul(st_all[32 * idx : 32 * idx + 2], sel2, sq,
                                 start=True, stop=True,
                                 tile_position=(0, 32 * idx))
                return ppc

            def rms_stage2(nk, g, st_all, ppcs):
                """one ln+exp over this group's (q,k) rms rows, then per-chain
                broadcast + fp8 normalize (Pool, all-SBUF).

                1/rms = exp(-0.5*ln(ms/D + eps)): ln and exp share the exp
                activation table, so the attention exp stream never reloads
                ACT tables (sqrt would force a reload per call)."""
                cs = slice(512 * nk, 512 * (nk + 1))
                base = 64 * g
                lnr = rot.tile([34, 512], bf16, tag="lnr", name="lnr", bufs=2)
                nc.scalar.activation(lnr, st_all[base : base + 34], AF.Ln,
                                     bias=eps_col[0:34], scale=1.0 / D)
                rinv = rot.tile([34, 512], bf16, tag="rinv", name="rinv",
                                bufs=2)
                nc.scalar.activation(rinv, lnr, AF.Exp, scale=-0.5)
                for nm in ("wq", "wk"):
                    idx = 2 * g + (1 if nm == "wk" else 0)
                    mm = psA.tile([128, 512], f32, tag="A", name="mm_ps")
                    nc.tensor.matmul(
                        mm, selT4[0:34, 128 * idx : 128 * (idx + 1)],
                        rinv, start=True, stop=True)
                    mrbc = rot.tile([128, 512], bf16, tag="mrbc",
                                    name="mrbc", bufs=2)
                    nc.vector.tensor_copy(mrbc, mm)
                    ppc = ppcs[idx]
                    if nm == "wq":
                        nc.gpsimd.tensor_tensor(
                            q8T[g][:, cs], ppc, mrbc, OP.mult)
                    else:
                        ktmp = rot.tile([128, 512], bf16, tag="ktmp",
                                        name="ktmp", bufs=2)
                        nc.gpsimd.tensor_tensor(
                            ktmp, ppc, mrbc, OP.mult)
                        nc.gpsimd.tensor_copy(kDR[g][:, 0, cs], ktmp)
                        nc.gpsimd.tensor_tensor(kDR[g][:, 1, cs], ktmp,
                                                kDR[g][:, 0, cs],
                                                OP.subtract)

            def v_epilogue(g, nk, pp):
                cs = slice(512 * nk, 512 * (nk + 1))
                nc.vector.tensor_copy(vT[g][:, cs], pp)
                for kb in range(4 * nk, 4 * nk + 4):
                    pv = psA.tile([128, 128], bf16, tag="A", name="vtr_ps")
                    nc.tensor.transpose(pv, vT[g][:, 128 * kb : 128 * (kb + 1)],
                                        ident)
                    dst = vaug[kb].rearrange("p (h c) -> p h c", h=HPC)[
                        :, 2 * g : 2 * g + 2, 0:64]
                    nc.vector.tensor_copy(
                        dst, pv.rearrange("p (h c) -> p h c", h=2))

            # ---- attention machinery --------------------------------------
            pending_oproj = []

            def emit_oproj(count=99):
                while pending_oproj and count > 0:
                    count -= 1
                    tb = pending_oproj.pop(0)
                    for nn in range(2):
                        po = psP.tile([128, 512], f32, tag="P", name="po_ps")
                        for cg in range(NG):
                            nc.tensor.matmul(
                                po,
                                AT[cg][:, 128 * tb : 128 * (tb + 1)],
                                wo_sb[:, 1024 * cg + 512 * nn :
                                      1024 * cg + 512 * (nn + 1)],
                                start=(cg == 0),
                                stop=(cg == NG - 1),
                            )
                        ob = obp.tile([128, 512], bf16, tag="ob", name="ob")
                        nc.vector.tensor_copy(ob, po)
                        nc.sync.dma_start(
                            out_d[128 * tb : 128 * (tb + 1),
                                  512 * nn : 512 * (nn + 1)],
                            ob,
                        )

            def attn_tile(j):
                """emit attention for q-tile j as a list of chunks; caller
                interleaves the chunks with projection chains."""
                entries = plan[j]
                if not entries:
                    return []
                nent = len(entries)
                last_for_qb = {}
                for ei, (kb, ql, qh, subs) in enumerate(entries):
                    for qbl in range(ql // 128, qh // 128):
                        last_for_qb[qbl] = ei
                pts = {}
                av_done = set()

                def emit_av(g, qbl):
                    av_done.add((g, qbl))
                    o_q = [psP.tile([128, 512], f32, tag="P", name="o_q")
                           for _ in range(2)]
                    started = False
                    for ei, (kb, ql, qh, subs) in enumerate(entries):
                        if not (ql <= 128 * qbl < qh):
                            continue
                        ptd, delta = pts[(g, ei)]
                        for hl in range(2):
                            h = 2 * g + hl
                            nc.tensor.matmul(
                                o_q[hl][:, 0:65],
                                ptd[:, 512 * hl + delta + 128 * qbl :
                                    512 * hl + delta + 128 * (qbl + 1)],
                                vaug[kb][:, 65 * h : 65 * (h + 1)],
                                start=not started,
                                stop=(ei == last_for_qb[qbl]),
                            )
                        started = True
                    minv2 = rot.tile([128, 2], f32, tag="mv", name="minv2",
                                     bufs=4)
                    for hl in range(2):
                        nc.vector.reciprocal(minv2[:, hl : hl + 1],
                                             o_q[hl][:, 64:65])
                    for hl in range(2):
                        nc.vector.scalar_tensor_tensor(
                            AT_q[qbl][:, 128 * g + 64 * hl :
                                      128 * g + 64 * (hl + 1)],
                            o_q[hl][:, 0:64],
                            minv2[:, hl : hl + 1],
                            vaug[4 * j + qbl][
                                :, 65 * (2 * g + hl) : 65 * (2 * g + hl) + 64],
                            OP.mult,
                            OP.add,
                        )
                    if j == QT - 1:
                        emit_oproj(1)
                    if g == NG - 1:
                        tb = 4 * j + qbl
                        if USE_DMA_TRANSPOSE and j < QT - 1:
                            for cg in range(NG):
                                nc.sync.dma_start_transpose(
                                    AT[cg][:, 128 * tb : 128 * (tb + 1)],
                                    AT_q[qbl][:, 128 * cg : 128 * (cg + 1)],
                                )
                        else:
                            for cg in range(NG):
                                tps = psA.tile([128, 128], bf16, tag="A",
                                               name="tps")
                                nc.tensor.transpose(
                                    tps, AT_q[qbl][:, 128 * cg : 128 * (cg + 1)],
                                    ident)
                                nc.vector.tensor_copy(
                                    AT[cg][:, 128 * tb : 128 * (tb + 1)], tps)
                        if j == QT - 1:
                            pending_oproj.append(tb)
                            emit_oproj(1)

                chunks = []
                av_cadence = max(1, nent // 4)

                def score_chunk(g, eis):
                    def run():
                        # pack consecutive taper entries (w1+w2 <= 512) into
                        # one S tile / one exp call
                        groups = []
                        i = 0
                        while i < len(eis):
                            ei = eis[i]
                            if i + 1 < len(eis):
                                e2 = eis[i + 1]
                                w1 = entries[ei][2] - entries[ei][1]
                                w2 = entries[e2][2] - entries[e2][1]
                                if w1 + w2 <= 512:
                                    groups.append([(ei, 0), (e2, w1)])
                                    i += 2
                                    continue
                            groups.append([(ei, None)])
                            i += 1
                        for group in groups:
                            s_ps = psS.tile([128, 1024], f32, tag="S",
                                            name="s_ps")
                            pt = ptp.tile([128, 1024], bf16, tag="PT",
                                          name="pt")
                            s3 = s_ps.rearrange("p (h w) -> p h w", h=2)
                            p3 = pt.rearrange("p (h w) -> p h w", h=2)
                            lo, hi = 512, 0
                            for ei, base in group:
                                kb, ql, qh, subs = entries[ei]
                                w = qh - ql
                                b = ql if base is None else base
                                lo, hi = min(lo, b), max(hi, b + w)
                                for hl in range(2):
                                    nc.tensor.matmul(
                                        s_ps[:, 512 * hl + b : 512 * hl + b + w],
                                        kDR[g][64 * hl : 64 * (hl + 1), :,
                                               128 * kb : 128 * (kb + 1)],
                                        q8T[g][64 * hl : 64 * (hl + 1),
                                               512 * j + ql : 512 * j + qh]
                                        .unsqueeze(1).broadcast_to([64, 2, w]),
                                        start=True,
                                        stop=True,
                                        perf_mode=DR,
                                        tile_position=(64 * hl, 0),
                                    )
                            nc.scalar.activation(
                                p3[:, :, lo:hi], s3[:, :, lo:hi], AF.Exp,
                                bias=nb_col, scale=SCALE)
                            for ei, base in group:
                                kb, ql, qh, subs = entries[ei]
                                b = ql if base is None else base
                                delta = b - ql
                                for hl in range(2):
                                    for qbl, pidx in subs:
                                        bs = slice(
                                            512 * hl + delta + 128 * qbl,
                                            512 * hl + delta + 128 * (qbl + 1))
                                        nc.gpsimd.tensor_tensor(
                                            pt[:, bs], pt[:, bs],
                                            pats[:, 128 * pidx :
                                                 128 * (pidx + 1)],
                                            OP.mult)
                                pts[(g, ei)] = (pt, delta)
                            ei = group[-1][0]
                            if j == QT - 1 and (g + ei) % 2 == 1:
                                emit_oproj(1)
                            if g == 1 and (ei + 1) % av_cadence == 0:
                                nqb = len([1 for x in av_done if x[0] == 0])
                                if nqb < 4:
                                    emit_av(0, nqb)
                    return run

                # split each g's entry stream into ~3 chunks for interleaving
                for g in range(NG):
                    eis = list(range(nent))
                    step = max(1, (nent + 3) // 4)
                    for s in range(0, nent, step):
                        chunks.append(score_chunk(g, eis[s : s + step]))

                def tail():
                    for g in range(NG):
                        for qbl in range(4):
                            if (g, qbl) not in av_done:
                                emit_av(g, qbl)
                    if j < QT - 1:
                        pending_oproj.extend(range(4 * j, 4 * j + 4))
                chunks.append(tail)
                return chunks

            # ---- pipelined emission ---------------------------------------
            prev_chunks = []

            def drain(n):
                for _ in range(n):
                    if prev_chunks:
                        prev_chunks.pop(0)()

            for nk in range(QT):
                dma_batch(nk)
                st_all = psA.tile([128, 512], f32, tag="A", name="st_all")
                nc.vector.memset(st_all[0:98], 1.0)
                ppcs = {}
                for g in range(NG):
                    for nm in ("wq", "wk"):
                        pp = proj_chain(nm, g, nk)
                        idx = 2 * g + (1 if nm == "wk" else 0)
                        ppcs[idx] = rms_stage1(nm, g, nk, pp, st_all)
                        drain(1)
                    rms_stage2(nk, g, st_all, ppcs)
                # flush the previous tile, then start this tile's score
                # stream before the v projections so the exp pipeline never
                # waits on them (v feeds only the later AV stage).
                drain(99)
                prev_chunks = attn_tile(nk)
                drain(2)
                for g in range(NG):
                    pp = proj_chain("wv", g, nk)
                    v_epilogue(g, nk, pp)
                    drain(1)
            drain(99)
            emit_oproj()

    nc.compile()
    return nc


def _pair_split(a):
    hi = a.astype(FP8)
    lo = (a - hi.astype(np.float32)).astype(FP8E5)
    return hi, lo


def _chunk_pair_x(xT):
    """[C, T] f32 -> two [128, C2*2*T] arrays (e4m3 body, e5m2 delta)."""
    hi, lo = _pair_split(xT)
    out = []
    for arr in (hi, lo):
        a = arr.reshape(C2, 2, 128, T).transpose(2, 0, 1, 3).reshape(128, -1)
        out.append(np.ascontiguousarray(a))
    return out


def _chunk_pair_w(W):
    """[C, 256] f32 -> two [128, C2*2*256] arrays."""
    hi, lo = _pair_split(W)
    out = []
    for arr in (hi, lo):
        a = arr.reshape(C2, 2, 128, 256).transpose(2, 0, 1, 3).reshape(128, -1)
        out.append(np.ascontiguousarray(a))
    return out


def kernel(**inputs):
    from concourse import bass_utils

    x = np.asarray(inputs["x"], np.float32)
    mask = np.asarray(inputs["attention_mask"])
    Wq = np.asarray(inputs["Wq"], np.float32)
    Wk = np.asarray(inputs["Wk"], np.float32)
    Wv = np.asarray(inputs["Wv"], np.float32)
    Wo = np.asarray(inputs["Wo"], np.float32)
    qw = np.asarray(inputs["q_norm_w"], np.float32)
    kw = np.asarray(inputs["k_norm_w"], np.float32)
    gate = np.asarray(inputs["gate"], np.float32).reshape(H)

    mask01 = mask.reshape(T, T) != 0
    plan, patterns = _analyze_mask(mask01)
    npat = patterns.shape[0]

    # fold the per-head gate into the value/output projections
    Wv = Wv / np.repeat(gate, D)[None, :]
    Wo = Wo * np.repeat(gate, D)[:, None]

    bound = 8.0 * np.max(np.abs(qw)) * np.max(np.abs(kw))
    neg_bias = -max(0.0, float(bound) - 60.0)

    key = (hash(mask01.tobytes()), npat, neg_bias)
    if key not in _CACHE:
        _CACHE[key] = _build_program(plan, npat, neg_bias)
    nc = _CACHE[key]

    pats_r = np.ascontiguousarray(
        patterns.transpose(1, 0, 2).reshape(128, 128 * npat)
    ).astype(BF16)
    sel2 = np.zeros((128, 2), np.float32)
    sel2[0:64, 0] = 1.0
    sel2[64:128, 1] = 1.0
    selT4 = np.zeros((34, 512), np.float32)
    for idx in range(4):
        w = qw if idx % 2 == 0 else kw
        r = 32 * (idx % 2)
        selT4[r + 0, 128 * idx : 128 * idx + 64] = w
        selT4[r + 1, 128 * idx + 64 : 128 * idx + 128] = w
    selT4 = selT4.astype(BF16)
    ident128 = np.eye(128, dtype=np.float32).astype(BF16)
    wq_col = np.tile(qw, 2)[:, None].astype(np.float32)
    wk_col = np.tile(kw, 2)[:, None].astype(np.float32)

    def chunk_major(W):
        ci, n = W.shape
        return np.ascontiguousarray(
            W.reshape(ci // 128, 128, n).transpose(1, 0, 2).reshape(128, -1)
        ).astype(BF16)

    in_maps = []
    for core in range(NCORES):
        b, grp = core // 4, core % 4
        hs = slice(256 * grp, 256 * (grp + 1))
        xT = np.ascontiguousarray(x[b].T)
        x8, xd = _chunk_pair_x(xT)
        m = {
            "x8": x8,
            "xd": xd,
            "wo": chunk_major(Wo[hs, :]),
            "wq_col": wq_col,
            "wk_col": wk_col,
            "sel2": sel2.astype(BF16),
            "selT4": selT4,
            "ident128": ident128,
            "pats": pats_r,
        }
        for nm, W in (("wq", Wq), ("wk", Wk), ("wv", Wv)):
            w8, wdl = _chunk_pair_w(W[:, hs])
            m[f"{nm}8"] = w8
            m[f"{nm}d"] = wdl
        in_maps.append(m)

    global _LAST_IN_MAPS
    _LAST_IN_MAPS = in_maps
    res = bass_utils.run_bass_kernel_spmd(nc, in_maps, core_ids=list(range(NCORES)))
    parts = [res.results[i]["out"].astype(np.float32) for i in range(NCORES)]
    out = np.stack(
        [
            parts[0] + parts[1] + parts[2] + parts[3],
            parts[4] + parts[5] + parts[6] + parts[7],
        ]
    )
    return out.astype(np.float32)
